# revision 1
# baseline (speedup 1.0000x reference)
# Graph-attention block (pre-LN, 4-head edge softmax, residual) on 8 Trainium2
# NeuronCores via Bass/Tile.
#
# Strategy (edge-cut partitioning per the sharding hint):
#   - Nodes are partitioned across the 8 cores by destination (1250 nodes/core,
#     padded to 1280 = 10 windows of 128).
#   - Each core computes LN1 + q/k/v projections for its own node slice; the
#     fp16 [k|v] rows are AllGathered so every core holds the full 10240x512
#     table, from which it bulk-gathers the source rows of its own edges.
#   - Edges are binned to the core owning their dst, sorted by dst, padded so
#     every (core, window) has the same tile count T; per 128-edge tile the
#     kernel builds one-hot matrices from the dst indices and uses the tensor
#     engine both to expand q rows per edge and to segment-sum the
#     exp-weighted v rows (plus the exp weights themselves as 4 extra columns,
#     giving the softmax normalizer z in the same matmul accumulation).
#   - Window epilogue divides by z, then the output projection + LN2 + ReLU +
#     residual runs per 128-node tile.
import math
from contextlib import ExitStack

import numpy as np

import concourse.bass as bass
import concourse.tile as tile
from concourse import bacc, mybir
from concourse.bass_utils import run_bass_kernel_spmd
from concourse.masks import make_identity

F32 = mybir.dt.float32
F16 = mybir.dt.float16
I16 = mybir.dt.int16
I32 = mybir.dt.int32
AF = mybir.ActivationFunctionType
ALU = mybir.AluOpType
AX = mybir.AxisListType

EPS = 1e-5
D = 256
H = 4
HD = 64
NCORE = 8


def _cdiv(a, b):
    return (a + b - 1) // b


def prep_inputs(x, edge_index, n_nodes):
    """Host-side edge binning/sorting/padding. Returns per-core arrays + T."""
    npc = n_nodes // NCORE            # real nodes per core
    nwin = _cdiv(npc, 128)            # 128-node windows per core
    npad = nwin * 128                 # padded nodes per core
    src = np.asarray(edge_index[0], dtype=np.int64)
    dst = np.asarray(edge_index[1], dtype=np.int64)

    per_core = []
    tiles = np.zeros((NCORE, nwin), dtype=np.int64)
    for c in range(NCORE):
        m = (dst // npc) == c
        s = src[m]
        dl = dst[m] - c * npc
        order = np.argsort(dl, kind="stable")
        s, dl = s[order], dl[order]
        w = dl // 128
        cnt = np.bincount(w, minlength=nwin)
        tiles[c] = np.maximum(_cdiv(cnt, 128), 1)
        per_core.append((s, dl, cnt))
    T = int(tiles.max())

    out = []
    for c in range(NCORE):
        s, dl, cnt = per_core[c]
        ne = nwin * T * 128
        src_pad = np.zeros(ne, dtype=np.int64)
        dadj_pad = np.full(ne, -1.0, dtype=np.float16)
        base = np.concatenate([[0], np.cumsum(cnt)])
        for w in range(nwin):
            seg = slice(base[w], base[w + 1])
            k = cnt[w]
            o = w * T * 128
            src_pad[o:o + k] = s[seg]
            dadj_pad[o:o + k] = (dl[seg] - 128 * w).astype(np.float16)
        # global row index in the padded AllGather table
        gidx = ((src_pad // npc) * npad + src_pad % npc).astype(np.int16)
        # dma_gather idx layout: per window block, idx j -> [j%16, j//16], x8 replicated
        blocks = []
        for w in range(nwin):
            b = gidx[w * T * 128:(w + 1) * T * 128].reshape(T * 8, 16).T
            blocks.append(np.tile(b, (8, 1)))
        kv_idx = np.ascontiguousarray(np.concatenate(blocks, axis=1))
        # edge-major dst one-hot source: [p, w*T+t] = dadj of edge (w, t, p)
        dadj_col = np.ascontiguousarray(
            dadj_pad.reshape(nwin * T, 128).T).astype(np.float16)
        xs = np.zeros((npad, D), dtype=np.float32)
        xs[:npc] = x[c * npc:(c + 1) * npc]
        out.append(dict(kv_idx=kv_idx, dadj_col=dadj_col, x_pad=xs))
    return out, T, nwin, npad, npc


def build_program(T, nwin, npad, flags, bench=False, skips=()):
    """Build the SPMD Bass program. flags: dict of skip_* bools."""
    V = NCORE * npad
    nc = bacc.Bacc("TRN2", target_bir_lowering=False, debug=False,
                   num_devices=NCORE)

    # ---- I/O ----
    x_ap = nc.dram_tensor("x_pad", [npad, D], F32, kind="ExternalInput").ap()
    wq_ap = nc.dram_tensor("wq", [D, D], F16, kind="ExternalInput").ap()
    wk_ap = nc.dram_tensor("wk", [D, D], F16, kind="ExternalInput").ap()
    wv_ap = nc.dram_tensor("wv", [D, D], F16, kind="ExternalInput").ap()
    wo_ap = nc.dram_tensor("wo", [D, D], F16, kind="ExternalInput").ap()
    vec_ap = nc.dram_tensor("vecs", [8, D], F32, kind="ExternalInput").ap()
    # vecs rows: 0:bq', 1:bk', 2:bv', 3:bo, 4:gamma2, 5:beta2 (fp32)
    kvidx_ap = nc.dram_tensor("kv_idx", [128, nwin * T * 8], I16,
                              kind="ExternalInput").ap()
    dadjc_ap = nc.dram_tensor("dadj_col", [128, nwin * T], F16,
                              kind="ExternalInput").ap()
    y_ap = nc.dram_tensor("y", [npad, D], F32, kind="ExternalOutput").ap()
    n_ap = (nc.dram_tensor("niter", [1, 1], I32, kind="ExternalInput").ap()
            if bench else None)

    kv_local = nc.dram_tensor("kv_local", [npad, 2 * D], F16)
    kv_shared = nc.dram_tensor("kv_shared", [V, 2 * D], F16, addr_space="Shared")
    kv_tbl = nc.dram_tensor("kv_tbl", [V, 2 * D], F16)

    with tile.TileContext(nc) as tc, ExitStack() as ctx:
        cp = ctx.enter_context(tc.tile_pool(name="const", bufs=1))
        wp = ctx.enter_context(tc.tile_pool(name="work", bufs=3))
        gp = ctx.enter_context(tc.tile_pool(name="gath", bufs=2))
        pp = ctx.enter_context(tc.tile_pool(name="ps", bufs=2, space="PSUM"))
        up = ctx.enter_context(tc.tile_pool(name="psu", bufs=2, space="PSUM"))

        # ---- constants ----
        ident = cp.tile([128, 128], F16)
        make_identity(nc, ident[:])
        iota_i = cp.tile([128, 128], I16)
        nc.gpsimd.iota(iota_i[:], pattern=[[1, 128]], channel_multiplier=0)
        iota_mat = cp.tile([128, 128], F16)
        nc.vector.tensor_copy(iota_mat[:], iota_i[:])
        eps_sb = cp.tile([128, 1], F32)
        nc.gpsimd.memset(eps_sb[:], EPS)

        wq_sb = cp.tile([128, 2, D], F16)
        wk_sb = cp.tile([128, 2, D], F16)
        wv_sb = cp.tile([128, 2, D], F16)
        wo_sb = cp.tile([128, 2, D], F16)
        for w_ap, w_sb in ((wq_ap, wq_sb), (wk_ap, wk_sb), (wv_ap, wv_sb),
                           (wo_ap, wo_sb)):
            nc.sync.dma_start(out=w_sb[:],
                              in_=w_ap.rearrange("(b k) n -> k b n", k=128))
        vec_sb = cp.tile([8, D], F32)
        nc.sync.dma_start(out=vec_sb[:], in_=vec_ap[:, :])
        bvec = {}
        for name, row in (("bq", 0), ("bk", 1), ("bv", 2), ("bo", 3),
                          ("g2", 4), ("b2", 5)):
            if not flags.get("skip_" + name, False):
                t = cp.tile([128, D], F32, tag="bc_" + name)
                nc.gpsimd.partition_broadcast(t[:], vec_sb[row:row + 1, :])
                bvec[name] = t

        kvidx_sb = cp.tile([128, nwin * T * 8], I16)
        nc.sync.dma_start(out=kvidx_sb[:], in_=kvidx_ap[:, :])
        dadjc_sb = cp.tile([128, nwin * T], F16)
        nc.sync.dma_start(out=dadjc_sb[:], in_=dadjc_ap[:, :])

        if bench:
            nn_t = cp.tile([1, 1], I32)
            nc.sync.dma_start(out=nn_t[:], in_=n_ap[:, :])
        x_sb = cp.tile([128, nwin, D], F32)
        q_sb = cp.tile([128, nwin, D], F16)
        agg_sb = cp.tile([128, nwin, D], F16)

        niter_reg = (nc.values_load(nn_t[:1, :1], min_val=0, max_val=1000000,
                                    skip_runtime_bounds_check=True)
                     if bench else None)

        def layer_norm_stats(src_ap, tag):
            """mean/rstd of [128, D] rows; returns (mean, rstd) [128,1] f32."""
            mean = wp.tile([128, 1], F32, tag=tag + "_m")
            nc.vector.reduce_sum(out=mean[:], in_=src_ap, axis=AX.X)
            nc.scalar.mul(out=mean[:], in_=mean[:], mul=1.0 / D)
            return mean

        def rstd_from(xc_ap, tag):
            sq = wp.tile([128, D], F16, tag=tag + "_sq")
            var = wp.tile([128, 1], F32, tag=tag + "_v")
            nc.scalar.activation(out=sq[:], in_=xc_ap, func=AF.Square,
                                 accum_out=var[:])
            s = wp.tile([128, 1], F32, tag=tag + "_s")
            nc.scalar.activation(out=s[:], in_=var[:], func=AF.Sqrt,
                                 scale=1.0 / D, bias=eps_sb[:, :1])
            rstd = wp.tile([128, 1], F32, tag=tag + "_r")
            nc.vector.reciprocal(rstd[:], s[:])
            return rstd

        # ---- phase 1: LN1 + projections on own slice ----
        def phase1():
          for w in range(nwin):
            xw = x_sb[:, w, :]
            nc.sync.dma_start(out=xw, in_=x_ap[w * 128:(w + 1) * 128, :])
            mean = layer_norm_stats(xw, "ln1")
            xc = wp.tile([128, D], F32, tag="xc")
            nc.vector.tensor_scalar_sub(xc[:], xw, mean[:, :1])
            rstd = rstd_from(xc[:], "ln1")
            xn = wp.tile([128, D], F16, tag="xn")
            nc.vector.tensor_scalar_mul(xn[:], xc[:], rstd[:, :1])

            xnT = wp.tile([128, 2, 128], F16, tag="xnT")
            for kh in range(2):
                pt = pp.tile([128, 128], F16, tag="psA")
                nc.tensor.transpose(out=pt[:], in_=xn[:, kh * 128:(kh + 1) * 128],
                                    identity=ident[:])
                nc.vector.tensor_copy(xnT[:, kh, :], pt[:])

            kv16 = wp.tile([128, 2 * D], F16, tag="kv16")
            for name, w_sb_, dst in (("bq", wq_sb, None), ("bk", wk_sb, kv16[:, :D]),
                                     ("bv", wv_sb, kv16[:, D:])):
                ps = pp.tile([128, D], F32, tag="psA")
                for kh in range(2):
                    nc.tensor.matmul(ps[:], lhsT=xnT[:, kh, :],
                                     rhs=w_sb_[:, kh, :],
                                     start=(kh == 0), stop=(kh == 1))
                tgt = q_sb[:, w, :] if dst is None else dst
                if name in bvec:
                    tf = wp.tile([128, D], F32, tag="pbias")
                    nc.vector.tensor_add(tf[:], ps[:], bvec[name][:])
                    nc.scalar.copy(out=tgt, in_=tf[:])
                else:
                    nc.scalar.copy(out=tgt, in_=ps[:])
            nc.sync.dma_start(out=kv_local[w * 128:(w + 1) * 128, :], in_=kv16[:])

        def table_copy():
          for i in range(V // 128):
            t = wp.tile([128, 2 * D], F16, tag="tblcp")
            nc.sync.dma_start(out=t[:], in_=kv_shared[i * 128:(i + 1) * 128, :])
            nc.sync.dma_start(out=kv_tbl[i * 128:(i + 1) * 128, :], in_=t[:])

        def phase34():
          for w in range(nwin):
            scores = wp.tile([128, T * 4], F32, tag="scores")
            e_s = wp.tile([128, T * 4], F16, tag="es")
            kv_g = gp.tile([128, T, 2 * D], F16, tag="kvg")
            if "gather" not in skips:
              nc.gpsimd.dma_gather(
                out_ap=kv_g[:], in_ap=kv_shared[:, :],
                idxs_ap=kvidx_sb[:, w * T * 8:(w + 1) * T * 8],
                num_idxs=T * 128, num_idxs_reg=T * 128, elem_size=2 * D,
                single_packet=False,
            )
            m_win = wp.tile([128, T, 128], F16, tag="mwin")
            for t in range(T):
                if "allA" in skips:
                    continue
                g = w * T + t
                nc.vector.tensor_tensor(
                    out=m_win[:, t, :],
                    in0=dadjc_sb[:, g:g + 1].to_broadcast([128, 128]),
                    in1=iota_mat[:], op=ALU.is_equal)
                if "loopA" in skips:
                    continue
                pmt = pp.tile([128, 128], F16, tag="psA")
                nc.tensor.transpose(out=pmt[:], in_=m_win[:, t, :],
                                    identity=ident[:])
                mt = wp.tile([128, 128], F16, tag="mt")
                nc.vector.tensor_copy(mt[:], pmt[:])
                ps_qe = pp.tile([128, D], F32, tag="psQ")
                nc.tensor.matmul(ps_qe[:], lhsT=mt[:], rhs=q_sb[:, w, :],
                                 start=True, stop=True)
                qe = wp.tile([128, D], F16, tag="qe16")
                nc.scalar.copy(out=qe[:], in_=ps_qe[:])
                prod = wp.tile([128, D], F16, tag="prod")
                nc.vector.tensor_mul(prod[:], qe[:], kv_g[:, t, :D])
                nc.vector.reduce_sum(
                    out=scores[:, t * 4:(t + 1) * 4],
                    in_=prod[:].rearrange("p (h d) -> p h d", d=HD), axis=AX.X)
            if "allA" in skips:
                continue
            nc.scalar.activation(out=e_s[:], in_=scores[:], func=AF.Exp,
                                 scale=1.0 / math.sqrt(HD))
            ps_u = up.tile([128, 2 * D + 8], F32, tag="u")
            for t in range(T):
                g = w * T + t
                wt = wp.tile([128, D + 4], F16, tag="wt")
                nc.vector.tensor_tensor(
                    out=wt[:, :D].rearrange("p (h d) -> p h d", d=HD),
                    in0=kv_g[:, t, D:].rearrange("p (h d) -> p h d", d=HD),
                    in1=e_s[:, t * 4:(t + 1) * 4].to_broadcast([128, H, HD]),
                    op=ALU.mult)
                nc.scalar.copy(out=wt[:, D:], in_=e_s[:, t * 4:(t + 1) * 4])
                nc.tensor.matmul(ps_u[:, :D + 4], lhsT=m_win[:, t, :], rhs=wt[:],
                                 start=(t == 0), stop=(t == T - 1))
            z = wp.tile([128, 4], F32, tag="z")
            nc.vector.tensor_scalar_add(z[:], ps_u[:, D:D + 4], 1e-30)
            rz = wp.tile([128, 4], F32, tag="rz")
            nc.vector.reciprocal(rz[:], z[:])
            nc.vector.tensor_tensor(
                out=agg_sb[:, w, :].rearrange("p (h d) -> p h d", d=HD),
                in0=ps_u[:, :D].rearrange("p (h d) -> p h d", d=HD),
                in1=rz[:].to_broadcast([128, H, HD]), op=ALU.mult)

          # ---- phase 4: output projection + LN2 + relu + residual ----
          for w in range(nwin):
            if "allA" in skips:
                continue
            aT = wp.tile([128, 2, 128], F16, tag="aT")
            for kh in range(2):
                pt = pp.tile([128, 128], F16, tag="psA")
                nc.tensor.transpose(out=pt[:], in_=agg_sb[:, w, kh * 128:(kh + 1) * 128],
                                    identity=ident[:])
                nc.vector.tensor_copy(aT[:, kh, :], pt[:])
            ps_o = pp.tile([128, D], F32, tag="psA")
            for kh in range(2):
                nc.tensor.matmul(ps_o[:], lhsT=aT[:, kh, :], rhs=wo_sb[:, kh, :],
                                 start=(kh == 0), stop=(kh == 1))
            o = wp.tile([128, D], F32, tag="o")
            if "bo" in bvec:
                nc.vector.tensor_add(o[:], ps_o[:], bvec["bo"][:])
            else:
                nc.vector.tensor_copy(o[:], ps_o[:])
            mean = layer_norm_stats(o[:], "ln2")
            oc = wp.tile([128, D], F32, tag="oc")
            nc.vector.tensor_scalar_sub(oc[:], o[:], mean[:, :1])
            rstd = rstd_from(oc[:], "ln2")
            on = wp.tile([128, D], F32, tag="on")
            nc.vector.tensor_scalar_mul(on[:], oc[:], rstd[:, :1])
            if "g2" in bvec:
                nc.vector.tensor_mul(on[:], on[:], bvec["g2"][:])
            if "b2" in bvec:
                nc.vector.tensor_add(on[:], on[:], bvec["b2"][:])
            r = wp.tile([128, D], F32, tag="r")
            nc.scalar.activation(out=r[:], in_=on[:], func=AF.Relu)
            yf = wp.tile([128, D], F32, tag="yf")
            nc.vector.tensor_add(yf[:], r[:], x_sb[:, w, :])
            nc.sync.dma_start(out=y_ap[w * 128:(w + 1) * 128, :], in_=yf[:])

        phase1()
        nc.gpsimd.collective_compute(
            "AllGather", ALU.bypass,
            replica_groups=[list(range(NCORE))],
            ins=[kv_local.ap().opt()], outs=[kv_shared.ap().opt()],
        )
        if bench:
            if "gather" in skips:
                for _ in range(2):
                    kvz = gp.tile([128, T, 2 * D], F16, tag="kvg")
                    nc.vector.tensor_copy(kvz[:].bitcast(I16), iota_i[:, :1].to_broadcast([128, T, 2 * D]))
            with tc.For_i(0, niter_reg, 1):
                if "phase1" not in skips:
                    phase1()
                if "copy" in skips:
                    table_copy()
                if "phase34" not in skips:
                    phase34()
        else:
            phase34()

    nc.compile()
    return nc


_CACHE = {}


def kernel(x, edge_index, gamma1, beta1, gamma2, beta2,
           Wq, bq, Wk, bk, Wv, bv, Wo, bo):
    x = np.asarray(x, dtype=np.float32)
    edge_index = np.asarray(edge_index)
    n_nodes = x.shape[0]
    per_core, T, nwin, npad, npc = prep_inputs(x, edge_index, n_nodes)

    g1 = np.asarray(gamma1, np.float32)
    b1 = np.asarray(beta1, np.float32)
    wq_p = (g1[:, None] * np.asarray(Wq, np.float32)).astype(np.float16)
    wk_p = (g1[:, None] * np.asarray(Wk, np.float32)).astype(np.float16)
    wv_p = (g1[:, None] * np.asarray(Wv, np.float32)).astype(np.float16)
    wo_p = np.asarray(Wo, np.float32).astype(np.float16)
    bq_p = b1 @ np.asarray(Wq, np.float32) + np.asarray(bq, np.float32)
    bk_p = b1 @ np.asarray(Wk, np.float32) + np.asarray(bk, np.float32)
    bv_p = b1 @ np.asarray(Wv, np.float32) + np.asarray(bv, np.float32)
    bo_ = np.asarray(bo, np.float32)
    g2 = np.asarray(gamma2, np.float32)
    b2 = np.asarray(beta2, np.float32)
    vecs = np.stack([bq_p, bk_p, bv_p, bo_, g2, b2, np.zeros_like(g2),
                     np.zeros_like(g2)]).astype(np.float32)
    flags = dict(
        skip_bq=not bq_p.any(), skip_bk=not bk_p.any(), skip_bv=not bv_p.any(),
        skip_bo=not bo_.any(), skip_g2=bool((g2 == 1).all()),
        skip_b2=not b2.any(),
    )

    key = (T, nwin, npad, tuple(sorted(flags.items())))
    if key not in _CACHE:
        _CACHE[key] = build_program(T, nwin, npad, flags)
    nc = _CACHE[key]

    in_maps = []
    for c in range(NCORE):
        pc = per_core[c]
        in_maps.append(dict(
            x_pad=pc["x_pad"], wq=wq_p, wk=wk_p, wv=wv_p, wo=wo_p, vecs=vecs,
            kv_idx=pc["kv_idx"], dadj_col=pc["dadj_col"],
        ))
    res = run_bass_kernel_spmd(nc, in_maps, core_ids=list(range(NCORE)))
    out = np.concatenate([res.results[c]["y"][:npc] for c in range(NCORE)], axis=0)
    return out.astype(np.float32)



# revision 5
# speedup vs baseline: 1.2100x; 1.2100x over previous
# Graph-attention block (pre-LN, 4-head edge softmax, residual) on 8 Trainium2
# NeuronCores via Bass/Tile.
#
# Strategy (edge-cut partitioning per the sharding hint):
#   - Nodes are partitioned across the 8 cores by destination (1250 nodes/core,
#     padded to 1280 = 10 windows of 128).
#   - Each core computes LN1 + q/k/v projections for its own node slice; the
#     fp16 [k|v] rows are AllGathered so every core holds the full 10240x512
#     table, from which it bulk-gathers the source rows of its own edges
#     (descriptor-bound: ~7.6ns per gathered row).
#   - Edges are binned to the core owning their dst, sorted by dst, padded so
#     every (core, window) has the same tile count T. Per window both one-hot
#     orientations ([edge,dst] for the segment-sum matmul and [dst,edge] for
#     the q-expansion matmul) are built in single batched vector compares
#     against iota patterns - no per-tile transposes.
#   - Loop A (scores): per 4-tile supertile, 4 q-expansion matmuls into PSUM
#     quarters, one bulk scalar PSUM->SBUF drain, one batched q*k product and
#     a pairwise-add fold tree ending in a f32 reduce.
#   - Loop B (aggregation): batched exp-weighted v rows (+ the exp weights as
#     4 extra columns giving the softmax normalizer z) accumulated over the
#     window by the tensor engine; epilogue divides by z; output projection +
#     LN2 + ReLU + residual run inline per window.
import math
from contextlib import ExitStack

import numpy as np

import concourse.bass as bass
import concourse.tile as tile
from concourse import bacc, mybir
from concourse.bass_utils import run_bass_kernel_spmd
from concourse.masks import make_identity

F32 = mybir.dt.float32
F16 = mybir.dt.float16
I16 = mybir.dt.int16
I32 = mybir.dt.int32
AF = mybir.ActivationFunctionType
ALU = mybir.AluOpType
AX = mybir.AxisListType

EPS = 1e-5
D = 256
H = 4
HD = 64
NCORE = 8


def _cdiv(a, b):
    return (a + b - 1) // b


def prep_inputs(x, edge_index, n_nodes):
    """Host-side edge binning/sorting/padding. Returns per-core arrays + T."""
    npc = n_nodes // NCORE            # real nodes per core
    nwin = _cdiv(npc, 128)            # 128-node windows per core
    npad = nwin * 128                 # padded nodes per core
    src = np.asarray(edge_index[0], dtype=np.int64)
    dst = np.asarray(edge_index[1], dtype=np.int64)

    per_core = []
    tiles = np.zeros((NCORE, nwin), dtype=np.int64)
    for c in range(NCORE):
        m = (dst // npc) == c
        s = src[m]
        dl = dst[m] - c * npc
        order = np.argsort(dl, kind="stable")
        s, dl = s[order], dl[order]
        w = dl // 128
        cnt = np.bincount(w, minlength=nwin)
        tiles[c] = np.maximum(_cdiv(cnt, 128), 1)
        per_core.append((s, dl, cnt))
    T = int(tiles.max())

    out = []
    for c in range(NCORE):
        s, dl, cnt = per_core[c]
        ne = nwin * T * 128
        src_pad = np.zeros(ne, dtype=np.int64)
        dadj_pad = np.full(ne, -1.0, dtype=np.float16)
        base = np.concatenate([[0], np.cumsum(cnt)])
        for w in range(nwin):
            seg = slice(base[w], base[w + 1])
            k = cnt[w]
            o = w * T * 128
            src_pad[o:o + k] = s[seg]
            dadj_pad[o:o + k] = (dl[seg] - 128 * w).astype(np.float16)
        # global row index in the padded AllGather table
        gidx = ((src_pad // npc) * npad + src_pad % npc).astype(np.int16)
        # dma_gather idx layout: per window block, idx j -> [j%16, j//16], x8 replicated
        blocks = []
        for w in range(nwin):
            b = gidx[w * T * 128:(w + 1) * T * 128].reshape(T * 8, 16).T
            blocks.append(np.tile(b, (8, 1)))
        kv_idx = np.ascontiguousarray(np.concatenate(blocks, axis=1))
        # dadj per edge, edge-on-partition layout: [e%128, w*T + t]
        dadj_col = np.ascontiguousarray(
            dadj_pad.reshape(nwin * T, 128).T).astype(np.float16)
        # dadj per edge, row layout for partition_broadcast: [1, w*T*128 + e]
        dadj_row = dadj_pad.reshape(1, ne).astype(np.float16)
        xs = np.zeros((npad, D), dtype=np.float32)
        xs[:npc] = x[c * npc:(c + 1) * npc]
        out.append(dict(kv_idx=kv_idx, dadj_col=dadj_col, dadj_row=dadj_row,
                        x_pad=xs))
    return out, T, nwin, npad, npc


def build_program(T, nwin, npad, flags, bench=False, skips=()):
    """Build the SPMD Bass program. flags: dict of skip_* bools."""
    V = NCORE * npad
    nc = bacc.Bacc("TRN2", target_bir_lowering=False, debug=False,
                   num_devices=NCORE)

    # supertile groups of up to 4 tiles
    ST = [(t0, min(4, T - t0)) for t0 in range(0, T, 4)]

    # ---- I/O ----
    x_ap = nc.dram_tensor("x_pad", [npad, D], F32, kind="ExternalInput").ap()
    wq_ap = nc.dram_tensor("wq", [D, D], F16, kind="ExternalInput").ap()
    wk_ap = nc.dram_tensor("wk", [D, D], F16, kind="ExternalInput").ap()
    wv_ap = nc.dram_tensor("wv", [D, D], F16, kind="ExternalInput").ap()
    wo_ap = nc.dram_tensor("wo", [D, D], F16, kind="ExternalInput").ap()
    vec_ap = nc.dram_tensor("vecs", [8, D], F32, kind="ExternalInput").ap()
    # vecs rows: 0:bq', 1:bk', 2:bv', 3:bo, 4:gamma2, 5:beta2 (fp32)
    kvidx_ap = nc.dram_tensor("kv_idx", [128, nwin * T * 8], I16,
                              kind="ExternalInput").ap()
    dadjc_ap = nc.dram_tensor("dadj_col", [128, nwin * T], F16,
                              kind="ExternalInput").ap()
    dadjr_ap = nc.dram_tensor("dadj_row", [1, nwin * T * 128], F16,
                              kind="ExternalInput").ap()
    y_ap = nc.dram_tensor("y", [npad, D], F32, kind="ExternalOutput").ap()
    n_ap = (nc.dram_tensor("niter", [1, 1], I32, kind="ExternalInput").ap()
            if bench else None)

    kv_local = nc.dram_tensor("kv_local", [npad, 2 * D], F16)
    kv_shared = nc.dram_tensor("kv_shared", [V, 2 * D], F16, addr_space="Shared")

    with tile.TileContext(nc) as tc, ExitStack() as ctx:
        cp = ctx.enter_context(tc.tile_pool(name="const", bufs=1))
        wp = ctx.enter_context(tc.tile_pool(name="work", bufs=2))
        mp = ctx.enter_context(tc.tile_pool(name="mask", bufs=2))
        gp = ctx.enter_context(tc.tile_pool(name="gath", bufs=2))
        pp = ctx.enter_context(tc.tile_pool(name="ps", bufs=2, space="PSUM"))
        up = ctx.enter_context(tc.tile_pool(name="psu", bufs=2, space="PSUM"))

        # ---- constants ----
        ident = cp.tile([128, 128], F16)
        make_identity(nc, ident[:])
        # stage the int iota in a gather-pool ring buffer (reused later)
        ii = gp.tile([128, T * 128], I16, tag="kvg")
        nc.gpsimd.iota(ii[:], pattern=[[0, T], [1, 128]], channel_multiplier=0)
        iota_col = cp.tile([128, T * 128], F16)
        nc.vector.tensor_copy(iota_col[:], ii[:])
        ip = cp.tile([128, 1], I16)
        nc.gpsimd.iota(ip[:], pattern=[[0, 1]], channel_multiplier=1)
        iota_part = cp.tile([128, 1], F16)
        nc.vector.tensor_copy(iota_part[:], ip[:])
        eps_sb = cp.tile([128, 1], F32)
        nc.gpsimd.memset(eps_sb[:], EPS)

        wq_sb = cp.tile([128, 2, D], F16)
        wk_sb = cp.tile([128, 2, D], F16)
        wv_sb = cp.tile([128, 2, D], F16)
        wo_sb = cp.tile([128, 2, D], F16)
        for w_ap, w_sb in ((wq_ap, wq_sb), (wk_ap, wk_sb), (wv_ap, wv_sb),
                           (wo_ap, wo_sb)):
            nc.sync.dma_start(out=w_sb[:],
                              in_=w_ap.rearrange("(b k) n -> k b n", k=128))
        vec_sb = cp.tile([8, D], F32)
        nc.sync.dma_start(out=vec_sb[:], in_=vec_ap[:, :])
        bvec = {}
        for name, row in (("bq", 0), ("bk", 1), ("bv", 2), ("bo", 3),
                          ("g2", 4), ("b2", 5)):
            if not flags.get("skip_" + name, False):
                t = cp.tile([128, D], F32, tag="bc_" + name)
                nc.gpsimd.partition_broadcast(t[:], vec_sb[row:row + 1, :])
                bvec[name] = t

        kvidx_sb = cp.tile([128, nwin * T * 8], I16)
        nc.sync.dma_start(out=kvidx_sb[:], in_=kvidx_ap[:, :])
        dadjc_sb = cp.tile([128, nwin * T], F16)
        nc.sync.dma_start(out=dadjc_sb[:], in_=dadjc_ap[:, :])

        if bench:
            nn_t = cp.tile([1, 1], I32)
            nc.sync.dma_start(out=nn_t[:], in_=n_ap[:, :])
        x_sb = cp.tile([128, nwin, D], F16)
        q_sb = cp.tile([128, nwin, D], F16)

        niter_reg = (nc.values_load(nn_t[:1, :1], min_val=0, max_val=1000000,
                                    skip_runtime_bounds_check=True)
                     if bench else None)

        def ln_rstd(xc_ap, tag):
            """rstd of centered rows [128, D]; scalar-engine heavy."""
            sq = wp.tile([128, D], F16, tag=tag + "_sq")
            var = wp.tile([128, 1], F32, tag=tag + "_v")
            nc.scalar.activation(out=sq[:], in_=xc_ap, func=AF.Square,
                                 accum_out=var[:])
            s = wp.tile([128, 1], F32, tag=tag + "_s")
            nc.scalar.activation(out=s[:], in_=var[:], func=AF.Sqrt,
                                 scale=1.0 / D, bias=eps_sb[:, :1])
            rstd = wp.tile([128, 1], F32, tag=tag + "_r")
            nc.vector.reciprocal(rstd[:], s[:])
            return rstd

        # ---- phase 1: LN1 + projections on own slice ----
        def phase1():
          for w in range(nwin):
            xw = x_sb[:, w, :]
            nc.gpsimd.dma_start(out=xw, in_=x_ap[w * 128:(w + 1) * 128, :])
            mean = wp.tile([128, 1], F32, tag="ln1_m")
            nc.vector.reduce_sum(out=mean[:], in_=xw, axis=AX.X)
            xc = wp.tile([128, D], F32, tag="xc")
            nc.vector.scalar_tensor_tensor(
                out=xc[:], in0=mean[:, :1].to_broadcast([128, D]),
                scalar=-1.0 / D, op0=ALU.mult, in1=xw, op1=ALU.add)
            rstd = ln_rstd(xc[:], "ln1")
            xn = wp.tile([128, D], F16, tag="xn")
            nc.vector.tensor_scalar_mul(xn[:], xc[:], rstd[:, :1])

            xnT = wp.tile([128, 2, 128], F16, tag="xnT")
            for kh in range(2):
                pt = pp.tile([128, 128], F16, tag="psA")
                nc.tensor.transpose(out=pt[:], in_=xn[:, kh * 128:(kh + 1) * 128],
                                    identity=ident[:])
                nc.scalar.copy(out=xnT[:, kh, :], in_=pt[:])

            kv16 = wp.tile([128, 2 * D], F16, tag="kv16")
            for name, w_sb_, dst in (("bq", wq_sb, None), ("bk", wk_sb, kv16[:, :D]),
                                     ("bv", wv_sb, kv16[:, D:])):
                ps = pp.tile([128, D], F32, tag="psA")
                for kh in range(2):
                    nc.tensor.matmul(ps[:], lhsT=xnT[:, kh, :],
                                     rhs=w_sb_[:, kh, :],
                                     start=(kh == 0), stop=(kh == 1))
                tgt = q_sb[:, w, :] if dst is None else dst
                if name in bvec:
                    tf = wp.tile([128, D], F32, tag="pbias")
                    nc.vector.tensor_add(tf[:], ps[:], bvec[name][:])
                    nc.scalar.copy(out=tgt, in_=tf[:])
                else:
                    nc.scalar.copy(out=tgt, in_=ps[:])
            nc.sync.dma_start(out=kv_local[w * 128:(w + 1) * 128, :], in_=kv16[:])

        def phase34():
          for w in range(nwin):
            # -- per-window inputs (DMA / pool engine) --
            dr = wp.tile([1, T * 128], F16, tag="dr")
            nc.sync.dma_start(
                out=dr[:], in_=dadjr_ap[:1, w * T * 128:(w + 1) * T * 128])
            dadj_bc = mp.tile([128, T * 128], F16, tag="bc")
            nc.gpsimd.partition_broadcast(dadj_bc[:], dr[:1, :])
            kv_g = gp.tile([128, T, 2 * D], F16, tag="kvg")
            n_idx = 128 if "gather" in skips else T * 128
            nc.gpsimd.dma_gather(
                out_ap=kv_g[:, :_cdiv(n_idx, 128), :], in_ap=kv_shared.ap()[:, :],
                idxs_ap=kvidx_sb[:, w * T * 8:w * T * 8 + n_idx // 16],
                num_idxs=n_idx, num_idxs_reg=n_idx, elem_size=2 * D,
                single_packet=False)
            # -- one-hot masks, both orientations, batched --
            m_win = mp.tile([128, T, 128], F16, tag="mw")
            nc.vector.tensor_tensor(
                out=m_win[:],
                in0=dadjc_sb[:, w * T:(w + 1) * T].to_broadcast([128, T, 128]),
                in1=iota_col[:].rearrange("p (t j) -> p t j", j=128),
                op=ALU.is_equal)
            mt = mp.tile([128, T * 128], F16, tag="mt")
            nc.vector.tensor_tensor(
                out=mt[:], in0=dadj_bc[:],
                in1=iota_part[:].to_broadcast([128, T * 128]), op=ALU.is_equal)

            scores = wp.tile([128, T * 4], F32, tag="sc")
            if "noA" in skips:
                nc.vector.memset(scores[:], 0.0)
            else:
              for (t0, nt) in ST:
                ps_qe = pp.tile([128, 4, D], F32, tag="psQ")
                for j in range(nt):
                    nc.tensor.matmul(ps_qe[:, j, :],
                                     lhsT=mt[:, (t0 + j) * 128:(t0 + j + 1) * 128],
                                     rhs=q_sb[:, w, :], start=True, stop=True)
                qe16 = wp.tile([128, 4, D], F16, tag="qe16")
                nc.scalar.copy(out=qe16[:, :nt, :], in_=ps_qe[:, :nt, :])
                prod = wp.tile([128, 4, D], F16, tag="prod")
                nc.vector.tensor_tensor(
                    out=prod[:, :nt, :], in0=qe16[:, :nt, :],
                    in1=kv_g[:, t0:t0 + nt, :D], op=ALU.mult)
                pv = prod[:].rearrange("p t (h d) -> p (t h) d", d=HD)
                nh = nt * 4
                f1 = wp.tile([128, 16, 32], F16, tag="f1")
                nc.vector.tensor_tensor(out=f1[:, :nh, :], in0=pv[:, :nh, 0:32],
                                        in1=pv[:, :nh, 32:64], op=ALU.add)
                f2 = wp.tile([128, 16, 16], F16, tag="f2")
                nc.vector.tensor_tensor(out=f2[:, :nh, :], in0=f1[:, :nh, 0:16],
                                        in1=f1[:, :nh, 16:32], op=ALU.add)
                f3 = wp.tile([128, 16, 8], F16, tag="f3")
                nc.vector.tensor_tensor(out=f3[:, :nh, :], in0=f2[:, :nh, 0:8],
                                        in1=f2[:, :nh, 8:16], op=ALU.add)
                nc.vector.reduce_sum(
                    out=scores[:, t0 * 4:t0 * 4 + nh].rearrange(
                        "p (th one) -> p th one", one=1),
                    in_=f3[:, :nh, :], axis=AX.X)
            e_s = wp.tile([128, T * 4], F16, tag="es")
            nc.scalar.activation(out=e_s[:], in_=scores[:], func=AF.Exp,
                                 scale=1.0 / math.sqrt(HD))

            ps_u = up.tile([128, D + 8], F32, tag="u")
            if "noB" in skips:
                continue
            for (t0, nt) in ST:
                wt4 = wp.tile([128, 4, D + 8], F16, tag="wt")
                nc.vector.tensor_tensor(
                    out=wt4[:, :nt, :D].rearrange("p t (h d) -> p t h d", d=HD),
                    in0=kv_g[:, t0:t0 + nt, D:].rearrange(
                        "p t (h d) -> p t h d", d=HD),
                    in1=e_s[:, t0 * 4:(t0 + nt) * 4].rearrange(
                        "p (t h) -> p t h", h=4).to_broadcast([128, nt, 4, HD]),
                    op=ALU.mult)
                nc.vector.tensor_copy(
                    wt4[:, :nt, D:D + 4],
                    e_s[:, t0 * 4:(t0 + nt) * 4].rearrange("p (t h) -> p t h", h=4))
                for j in range(nt):
                    t = t0 + j
                    nc.tensor.matmul(ps_u[:, :D + 4], lhsT=m_win[:, t, :],
                                     rhs=wt4[:, j, :D + 4],
                                     start=(t == 0), stop=(t == T - 1))
            z = wp.tile([128, 4], F32, tag="z")
            nc.vector.tensor_scalar_add(z[:], ps_u[:, D:D + 4], 1e-30)
            rz = wp.tile([128, 4], F32, tag="rz")
            nc.vector.reciprocal(rz[:], z[:])
            aggt = wp.tile([128, D], F16, tag="aggt")
            nc.scalar.copy(out=aggt[:], in_=ps_u[:, :D])
            aggn = wp.tile([128, D], F16, tag="aggn")
            nc.vector.tensor_tensor(
                out=aggn[:].rearrange("p (h d) -> p h d", d=HD),
                in0=aggt[:].rearrange("p (h d) -> p h d", d=HD),
                in1=rz[:].to_broadcast([128, H, HD]), op=ALU.mult)

            # -- phase 4 inline: output projection + LN2 + relu + residual --
            aT = wp.tile([128, 2, 128], F16, tag="aT")
            for kh in range(2):
                pt = pp.tile([128, 128], F16, tag="psA")
                nc.tensor.transpose(out=pt[:], in_=aggn[:, kh * 128:(kh + 1) * 128],
                                    identity=ident[:])
                nc.scalar.copy(out=aT[:, kh, :], in_=pt[:])
            ps_o = pp.tile([128, D], F32, tag="psA")
            for kh in range(2):
                nc.tensor.matmul(ps_o[:], lhsT=aT[:, kh, :], rhs=wo_sb[:, kh, :],
                                 start=(kh == 0), stop=(kh == 1))
            o = wp.tile([128, D], F32, tag="o")
            if "bo" in bvec:
                nc.vector.tensor_add(o[:], ps_o[:], bvec["bo"][:])
            else:
                nc.scalar.copy(out=o[:], in_=ps_o[:])
            mean2 = wp.tile([128, 1], F32, tag="ln2_m")
            nc.vector.reduce_sum(out=mean2[:], in_=o[:], axis=AX.X)
            oc = wp.tile([128, D], F32, tag="oc")
            nc.vector.scalar_tensor_tensor(
                out=oc[:], in0=mean2[:, :1].to_broadcast([128, D]),
                scalar=-1.0 / D, op0=ALU.mult, in1=o[:], op1=ALU.add)
            rstd2 = ln_rstd(oc[:], "ln2")
            on = wp.tile([128, D], F32, tag="on")
            nc.vector.tensor_scalar_mul(on[:], oc[:], rstd2[:, :1])
            if "g2" in bvec:
                nc.vector.tensor_mul(on[:], on[:], bvec["g2"][:])
            if "b2" in bvec:
                nc.vector.tensor_add(on[:], on[:], bvec["b2"][:])
            r = wp.tile([128, D], F32, tag="r")
            nc.scalar.activation(out=r[:], in_=on[:], func=AF.Relu)
            yf = wp.tile([128, D], F32, tag="yf")
            nc.vector.tensor_add(yf[:], r[:], x_sb[:, w, :])
            nc.sync.dma_start(out=y_ap[w * 128:(w + 1) * 128, :], in_=yf[:])

        phase1()
        nc.gpsimd.collective_compute(
            "AllGather", ALU.bypass,
            replica_groups=[list(range(NCORE))],
            ins=[kv_local.ap().opt()], outs=[kv_shared.ap().opt()],
        )
        if bench:
            with tc.For_i(0, niter_reg, 1):
                if "phase1" not in skips:
                    phase1()
                if "phase34" not in skips:
                    phase34()
        else:
            phase34()

    nc.compile()
    return nc


_CACHE = {}


def kernel(x, edge_index, gamma1, beta1, gamma2, beta2,
           Wq, bq, Wk, bk, Wv, bv, Wo, bo):
    x = np.asarray(x, dtype=np.float32)
    edge_index = np.asarray(edge_index)
    n_nodes = x.shape[0]
    per_core, T, nwin, npad, npc = prep_inputs(x, edge_index, n_nodes)

    g1 = np.asarray(gamma1, np.float32)
    b1 = np.asarray(beta1, np.float32)
    wq_p = (g1[:, None] * np.asarray(Wq, np.float32)).astype(np.float16)
    wk_p = (g1[:, None] * np.asarray(Wk, np.float32)).astype(np.float16)
    wv_p = (g1[:, None] * np.asarray(Wv, np.float32)).astype(np.float16)
    wo_p = np.asarray(Wo, np.float32).astype(np.float16)
    bq_p = b1 @ np.asarray(Wq, np.float32) + np.asarray(bq, np.float32)
    bk_p = b1 @ np.asarray(Wk, np.float32) + np.asarray(bk, np.float32)
    bv_p = b1 @ np.asarray(Wv, np.float32) + np.asarray(bv, np.float32)
    bo_ = np.asarray(bo, np.float32)
    g2 = np.asarray(gamma2, np.float32)
    b2 = np.asarray(beta2, np.float32)
    vecs = np.stack([bq_p, bk_p, bv_p, bo_, g2, b2, np.zeros_like(g2),
                     np.zeros_like(g2)]).astype(np.float32)
    flags = dict(
        skip_bq=not bq_p.any(), skip_bk=not bk_p.any(), skip_bv=not bv_p.any(),
        skip_bo=not bo_.any(), skip_g2=bool((g2 == 1).all()),
        skip_b2=not b2.any(),
    )

    key = (T, nwin, npad, tuple(sorted(flags.items())))
    if key not in _CACHE:
        _CACHE[key] = build_program(T, nwin, npad, flags)
    nc = _CACHE[key]

    in_maps = []
    for c in range(NCORE):
        pc = per_core[c]
        in_maps.append(dict(
            x_pad=pc["x_pad"], wq=wq_p, wk=wk_p, wv=wv_p, wo=wo_p, vecs=vecs,
            kv_idx=pc["kv_idx"], dadj_col=pc["dadj_col"],
            dadj_row=pc["dadj_row"],
        ))
    res = run_bass_kernel_spmd(nc, in_maps, core_ids=list(range(NCORE)))
    out = np.concatenate([res.results[c]["y"][:npc] for c in range(NCORE)], axis=0)
    return out.astype(np.float32)


# revision 31
# speedup vs baseline: 1.4871x; 1.2290x over previous
# Graph-attention block (pre-LN, 4-head edge softmax, residual) on 8 Trainium2
# NeuronCores via Bass/Tile.
#
# Strategy (edge-cut partitioning per the sharding hint):
#   - Nodes are partitioned across the 8 cores by destination (1250 nodes/core,
#     padded to 1280 = 10 windows of 128).
#   - Each core computes LN1 + q/k/v projections for its own node slice; the
#     fp16 [k|v] rows are AllGathered so every core holds the full 10240x512
#     table, from which it bulk-gathers the source rows of its own edges
#     (descriptor-bound: ~7.6ns per gathered row).
#   - Edges are binned to the core owning their dst, sorted by dst, padded so
#     every (core, window) has the same tile count T. Per window both one-hot
#     orientations ([edge,dst] for the segment-sum matmul and [dst,edge] for
#     the q-expansion matmul) are built in single batched vector compares
#     against iota patterns - no per-tile transposes.
#   - Loop A (scores): per 4-tile supertile, 4 q-expansion matmuls into PSUM
#     quarters, one bulk scalar PSUM->SBUF drain, one batched q*k product and
#     a pairwise-add fold tree ending in a f32 reduce.
#   - Loop B (aggregation): batched exp-weighted v rows (+ the exp weights as
#     4 extra columns giving the softmax normalizer z) accumulated over the
#     window by the tensor engine; epilogue divides by z; output projection +
#     LN2 + ReLU + residual run inline per window.
import math
from contextlib import ExitStack

import numpy as np

import concourse.bass as bass
import concourse.tile as tile
from concourse import bacc, mybir
from concourse.bass_utils import run_bass_kernel_spmd
from concourse.masks import make_identity

F32 = mybir.dt.float32
F16 = mybir.dt.float16
I16 = mybir.dt.int16
I32 = mybir.dt.int32
AF = mybir.ActivationFunctionType
ALU = mybir.AluOpType
AX = mybir.AxisListType

EPS = 1e-5
D = 256
H = 4
HD = 64
NCORE = 8
REDUCE_MODE = "tree"   # "pool" | "tree"


def _cdiv(a, b):
    return (a + b - 1) // b


def prep_inputs(x, edge_index, n_nodes):
    """Host-side edge binning/sorting/padding. Returns per-core arrays + T."""
    npc = n_nodes // NCORE            # real nodes per core
    nwin = _cdiv(npc, 128)            # 128-node windows per core
    npad = nwin * 128                 # padded nodes per core
    src = np.asarray(edge_index[0], dtype=np.int64)
    dst = np.asarray(edge_index[1], dtype=np.int64)

    per_core = []
    tiles = np.zeros((NCORE, nwin), dtype=np.int64)
    for c in range(NCORE):
        m = (dst // npc) == c
        s = src[m]
        dl = dst[m] - c * npc
        order = np.argsort(dl, kind="stable")
        s, dl = s[order], dl[order]
        w = dl // 128
        cnt = np.bincount(w, minlength=nwin)
        tiles[c] = np.maximum(_cdiv(cnt, 128), 1)
        per_core.append((s, dl, cnt))
    T = int(tiles.max())

    out = []
    for c in range(NCORE):
        s, dl, cnt = per_core[c]
        ne = nwin * T * 128
        src_pad = np.zeros(ne, dtype=np.int64)
        dadj_pad = np.full(ne, -1.0, dtype=np.float16)
        base = np.concatenate([[0], np.cumsum(cnt)])
        for w in range(nwin):
            seg = slice(base[w], base[w + 1])
            k = cnt[w]
            o = w * T * 128
            src_pad[o:o + k] = s[seg]
            dadj_pad[o:o + k] = (dl[seg] - 128 * w).astype(np.float16)
        # global row index in the padded AllGather table
        gidx = ((src_pad // npc) * npad + src_pad % npc).astype(np.int16)
        # dma_gather idx layout: per window block, idx j -> [j%16, j//16], x8 replicated
        blocks = []
        for w in range(nwin):
            b = gidx[w * T * 128:(w + 1) * T * 128].reshape(T * 8, 16).T
            blocks.append(np.tile(b, (8, 1)))
        kv_idx = np.ascontiguousarray(np.concatenate(blocks, axis=1))
        # dadj per edge, edge-on-partition layout: [e%128, w*T + t]
        dadj_col = np.ascontiguousarray(
            dadj_pad.reshape(nwin * T, 128).T).astype(np.float16)
        # dadj per edge, row layout for partition_broadcast: [1, w*T*128 + e]
        dadj_row = dadj_pad.reshape(1, ne).astype(np.float16)
        xs = np.zeros((npad, D), dtype=np.float16)
        xs[:npc] = x[c * npc:(c + 1) * npc]
        out.append(dict(kv_idx=kv_idx, dadj_col=dadj_col, dadj_row=dadj_row,
                        x_pad=xs))
    return out, T, nwin, npad, npc


def build_program(T, nwin, npad, flags, bench=False, skips=()):
    """Build the SPMD Bass program. flags: dict of skip_* bools."""
    V = NCORE * npad
    nc = bacc.Bacc("TRN2", target_bir_lowering=False, debug=False,
                   num_devices=NCORE)

    # window split into two gather halves; supertile groups of up to 4 tiles
    # within each half (half boundary chosen 4-aligned so group count stays 9)
    H0 = (T // 2) // 4 * 4
    HALVES = [(0, H0), (H0, T - H0)]
    def _groups(n):
        return [(t0, min(4, n - t0)) for t0 in range(0, n, 4)]

    # ---- I/O ----
    x_ap = nc.dram_tensor("x_pad", [npad, D], F16, kind="ExternalInput").ap()
    wq_ap = nc.dram_tensor("wq", [D, D], F16, kind="ExternalInput").ap()
    wk_ap = nc.dram_tensor("wk", [D, D], F16, kind="ExternalInput").ap()
    wv_ap = nc.dram_tensor("wv", [D, D], F16, kind="ExternalInput").ap()
    wo_ap = nc.dram_tensor("wo", [D, D], F16, kind="ExternalInput").ap()
    vec_ap = nc.dram_tensor("vecs", [8, D], F32, kind="ExternalInput").ap()
    # vecs rows: 0:bq', 1:bk', 2:bv', 3:bo, 4:gamma2, 5:beta2 (fp32)
    kvidx_ap = nc.dram_tensor("kv_idx", [128, nwin * T * 8], I16,
                              kind="ExternalInput").ap()
    dadjc_ap = nc.dram_tensor("dadj_col", [128, nwin * T], F16,
                              kind="ExternalInput").ap()
    dadjr_ap = nc.dram_tensor("dadj_row", [1, nwin * T * 128], F16,
                              kind="ExternalInput").ap()
    y_ap = nc.dram_tensor("y", [npad, D], F32, kind="ExternalOutput").ap()
    n_ap = (nc.dram_tensor("niter", [1, 1], I32, kind="ExternalInput").ap()
            if bench else None)

    kv_local = nc.dram_tensor("kv_local", [npad, 2 * D], F16)
    kv_shared = nc.dram_tensor("kv_shared", [V, 2 * D], F16, addr_space="Shared")

    with tile.TileContext(nc) as tc, ExitStack() as ctx:
        cp = ctx.enter_context(tc.tile_pool(name="const", bufs=1))
        wp = ctx.enter_context(tc.tile_pool(name="work", bufs=2))
        mp = ctx.enter_context(tc.tile_pool(name="mask", bufs=2))
        bp = ctx.enter_context(tc.tile_pool(name="bcast", bufs=1))
        gp = ctx.enter_context(tc.tile_pool(name="gath", bufs=2))
        pp = ctx.enter_context(tc.tile_pool(name="ps", bufs=2, space="PSUM"))
        up = ctx.enter_context(tc.tile_pool(name="psu", bufs=2, space="PSUM"))

        # ---- constants ----
        ident = cp.tile([128, 128], F16)
        make_identity(nc, ident[:])
        # stage the int iota in a gather-pool ring buffer (reused later)
        ii = gp.tile([128, T * 128], I16, tag="kvg")
        nc.gpsimd.iota(ii[:], pattern=[[0, T], [1, 128]], channel_multiplier=0)
        iota_col = cp.tile([128, T * 128], F16)
        nc.vector.tensor_copy(iota_col[:], ii[:])
        ip = cp.tile([128, 1], I16)
        nc.gpsimd.iota(ip[:], pattern=[[0, 1]], channel_multiplier=1)
        iota_part = cp.tile([128, 1], F16)
        nc.vector.tensor_copy(iota_part[:], ip[:])
        eps_sb = cp.tile([128, 1], F32)
        nc.gpsimd.memset(eps_sb[:], EPS)

        wq_sb = cp.tile([128, 2, D], F16)
        wk_sb = cp.tile([128, 2, D], F16)
        wv_sb = cp.tile([128, 2, D], F16)
        wo_sb = cp.tile([128, 2, D], F16)
        for w_ap, w_sb in ((wq_ap, wq_sb), (wk_ap, wk_sb), (wv_ap, wv_sb),
                           (wo_ap, wo_sb)):
            nc.sync.dma_start(out=w_sb[:],
                              in_=w_ap.rearrange("(b k) n -> k b n", k=128))
        vec_sb = cp.tile([8, D], F32)
        nc.sync.dma_start(out=vec_sb[:], in_=vec_ap[:, :])
        bvec = {}
        for name, row in (("bq", 0), ("bk", 1), ("bv", 2), ("bo", 3),
                          ("g2", 4), ("b2", 5)):
            if not flags.get("skip_" + name, False):
                t = cp.tile([128, D], F32, tag="bc_" + name)
                nc.gpsimd.partition_broadcast(t[:], vec_sb[row:row + 1, :])
                bvec[name] = t

        kvidx_sb = cp.tile([128, nwin * T * 8], I16)
        nc.sync.dma_start(out=kvidx_sb[:], in_=kvidx_ap[:, :])
        dadjc_sb = cp.tile([128, nwin * T], F16)
        nc.sync.dma_start(out=dadjc_sb[:], in_=dadjc_ap[:, :])

        if bench:
            nn_t = cp.tile([1, 1], I32)
            nc.sync.dma_start(out=nn_t[:], in_=n_ap[:, :])
        x_sb = cp.tile([128, nwin, D], F16)
        q_sb = cp.tile([128, nwin, D], F16)
        agg_sb = cp.tile([128, nwin, D], F16)

        niter_reg = (nc.values_load(nn_t[:1, :1], min_val=0, max_val=1000000,
                                    skip_runtime_bounds_check=True)
                     if bench else None)

        def ln_mean(x_ap, tag):
            """Row-sum via scalar Copy+accum (no act-table switch)."""
            ms = wp.tile([128, D], F16, tag=tag + "_ms")
            mean = wp.tile([128, 1], F32, tag=tag + "_m")
            nc.scalar.activation(out=ms[:], in_=x_ap, func=AF.Copy,
                                 accum_out=mean[:])
            return mean

        def ln_rstd(xc_ap, tag):
            """rstd of centered rows [128, D]; scalar Square+Sqrt (both live
            in the sqrt act-table set together with Copy/Relu)."""
            sq = wp.tile([128, D], F16, tag=tag + "_sq")
            var = wp.tile([128, 1], F32, tag=tag + "_v")
            nc.scalar.activation(out=sq[:], in_=xc_ap, func=AF.Square,
                                 accum_out=var[:])
            s = wp.tile([128, 1], F32, tag=tag + "_s")
            nc.scalar.activation(out=s[:], in_=var[:], func=AF.Sqrt,
                                 scale=1.0 / D, bias=eps_sb[:, :1])
            rstd = wp.tile([128, 1], F32, tag=tag + "_r")
            nc.vector.reciprocal(rstd[:], s[:])
            return rstd

        # ---- phase 1: LN1 + projections on own slice ----
        def phase1():
          for w in range(nwin):
            xw = x_sb[:, w, :]
            nc.sync.dma_start(out=xw, in_=x_ap[w * 128:(w + 1) * 128, :])
            mean = ln_mean(xw, "ln1")
            xc = wp.tile([128, D], F32, tag="xc")
            nc.vector.scalar_tensor_tensor(
                out=xc[:], in0=mean[:, :1].to_broadcast([128, D]),
                scalar=-1.0 / D, op0=ALU.mult, in1=xw, op1=ALU.add)
            rstd = ln_rstd(xc[:], "ln1")
            xn = wp.tile([128, D], F16, tag="xn")
            nc.vector.tensor_scalar_mul(xn[:], xc[:], rstd[:, :1])

            xnT = wp.tile([128, 2, 128], F16, tag="xnT")
            for kh in range(2):
                pt = pp.tile([128, 128], F16, tag="psA")
                nc.tensor.transpose(out=pt[:], in_=xn[:, kh * 128:(kh + 1) * 128],
                                    identity=ident[:])
                nc.scalar.copy(out=xnT[:, kh, :], in_=pt[:])

            kv16 = wp.tile([128, 2 * D], F16, tag="kv16")
            for name, w_sb_, dst in (("bq", wq_sb, None), ("bk", wk_sb, kv16[:, :D]),
                                     ("bv", wv_sb, kv16[:, D:])):
                ps = pp.tile([128, D], F32, tag="psA")
                for kh in range(2):
                    nc.tensor.matmul(ps[:], lhsT=xnT[:, kh, :],
                                     rhs=w_sb_[:, kh, :],
                                     start=(kh == 0), stop=(kh == 1))
                tgt = q_sb[:, w, :] if dst is None else dst
                if name in bvec:
                    tf = wp.tile([128, D], F32, tag="pbias")
                    nc.vector.tensor_add(tf[:], ps[:], bvec[name][:])
                    nc.scalar.copy(out=tgt, in_=tf[:])
                else:
                    nc.scalar.copy(out=tgt, in_=ps[:])
            nc.sync.dma_start(out=kv_local[w * 128:(w + 1) * 128, :], in_=kv16[:])

        def phase34():
          for w in range(nwin):
            # -- per-window inputs (DMA / pool engine) --
            # gathers FIRST on the in-order Pool queue so they are never
            # stalled behind a broadcast that waits on DVE mask builds
            kv_g = gp.tile([128, T, 2 * D], F16, tag="kvg")
            n_idx = 128 if "gather" in skips else T * 128
            nc.gpsimd.dma_gather(
                out_ap=kv_g[:, :_cdiv(n_idx, 128), :],
                in_ap=kv_shared.ap()[:, :],
                idxs_ap=kvidx_sb[:, w * T * 8:w * T * 8 + n_idx // 16],
                num_idxs=n_idx, num_idxs_reg=n_idx, elem_size=2 * D,
                single_packet=False)
            kv_h = [kv_g[:, o:o + n, :] for (o, n) in HALVES]
            dr = bp.tile([1, T * 128], F16, tag="dr")
            nc.sync.dma_start(
                out=dr[:], in_=dadjr_ap[:1, w * T * 128:(w + 1) * T * 128])
            dadj_bc = bp.tile([128, T * 128], F16, tag="bc")
            nc.gpsimd.partition_broadcast(dadj_bc[:], dr[:1, :])
            # -- one-hot masks, both orientations, batched --
            m_win = mp.tile([128, T, 128], F16, tag="mw")
            nc.vector.tensor_tensor(
                out=m_win[:],
                in0=dadjc_sb[:, w * T:(w + 1) * T].to_broadcast([128, T, 128]),
                in1=iota_col[:].rearrange("p (t j) -> p t j", j=128),
                op=ALU.is_equal)
            mt = mp.tile([128, T * 128], F16, tag="mt")
            nc.vector.tensor_tensor(
                out=mt[:], in0=dadj_bc[:],
                in1=iota_part[:].to_broadcast([128, T * 128]), op=ALU.is_equal)

            scores = wp.tile([128, T * 4], F32, tag="sc")
            if "noA" in skips:
                nc.vector.memset(scores[:], 0.0)
            else:
              for hi, (off, nt_h) in enumerate(HALVES):
               for (tl, nt) in _groups(nt_h):
                t0 = off + tl
                ps_qe = pp.tile([128, 4, D], F32, tag="psQ")
                for j in range(nt):
                    nc.tensor.matmul(ps_qe[:, j, :],
                                     lhsT=mt[:, (t0 + j) * 128:(t0 + j + 1) * 128],
                                     rhs=q_sb[:, w, :], start=True, stop=True)
                qe16 = wp.tile([128, 4, D], F16, tag="qe16")
                nc.scalar.copy(out=qe16[:, :nt, :], in_=ps_qe[:, :nt, :])
                prod = wp.tile([128, 4, D], F16, tag="prod")
                nc.vector.tensor_tensor(
                    out=prod[:, :nt, :], in0=qe16[:, :nt, :],
                    in1=kv_h[hi][:, tl:tl + nt, :D], op=ALU.mult)
                pv = prod[:].rearrange("p t (h d) -> p (t h) d", d=HD)
                nh = nt * 4
                if REDUCE_MODE == "pool":
                    nc.vector.pool(out=scores[:, t0 * 4:t0 * 4 + nh],
                                   in_=pv[:, :nh, :],
                                   func=mybir.PoolFunctionType.avg)
                elif REDUCE_MODE == "gp":
                    nc.gpsimd.reduce_sum(
                        out=scores[:, t0 * 4:t0 * 4 + nh].rearrange(
                            "p (th one) -> p th one", one=1),
                        in_=pv[:, :nh, :], axis=AX.X)
                else:
                    f1 = wp.tile([128, 16, 32], F16, tag="f1")
                    nc.vector.tensor_tensor(out=f1[:, :nh, :],
                                            in0=pv[:, :nh, 0:32],
                                            in1=pv[:, :nh, 32:64], op=ALU.add)
                    f2 = wp.tile([128, 16, 16], F16, tag="f2")
                    nc.vector.tensor_tensor(out=f2[:, :nh, :],
                                            in0=f1[:, :nh, 0:16],
                                            in1=f1[:, :nh, 16:32], op=ALU.add)
                    f3 = wp.tile([128, 16, 8], F16, tag="f3")
                    nc.vector.tensor_tensor(out=f3[:, :nh, :],
                                            in0=f2[:, :nh, 0:8],
                                            in1=f2[:, :nh, 8:16], op=ALU.add)
                    nc.vector.reduce_sum(
                        out=scores[:, t0 * 4:t0 * 4 + nh].rearrange(
                            "p (th one) -> p th one", one=1),
                        in_=f3[:, :nh, :], axis=AX.X)
            e_s = wp.tile([128, T * 4], F16, tag="es")
            # pool averages over HD; fold the *HD back into the exp scale
            es_scale = (float(HD) if REDUCE_MODE == "pool" else 1.0) / math.sqrt(HD)
            nc.scalar.activation(out=e_s[:], in_=scores[:], func=AF.Exp,
                                 scale=es_scale)

            ps_u = up.tile([128, D + 8], F32, tag="u")
            if "noB" in skips:
                continue
            for hi, (off, nt_h) in enumerate(HALVES):
              for (tl, nt) in _groups(nt_h):
                t0 = off + tl
                wt4 = wp.tile([128, 4, D + 8], F16, tag="wt")
                nc.vector.tensor_tensor(
                    out=wt4[:, :nt, :D].rearrange("p t (h d) -> p t h d", d=HD),
                    in0=kv_h[hi][:, tl:tl + nt, D:].rearrange(
                        "p t (h d) -> p t h d", d=HD),
                    in1=e_s[:, t0 * 4:(t0 + nt) * 4].rearrange(
                        "p (t h) -> p t h", h=4).to_broadcast([128, nt, 4, HD]),
                    op=ALU.mult)
                nc.vector.tensor_copy(
                    wt4[:, :nt, D:D + 4],
                    e_s[:, t0 * 4:(t0 + nt) * 4].rearrange("p (t h) -> p t h", h=4))
                for j in range(nt):
                    t = t0 + j
                    nc.tensor.matmul(ps_u[:, :D + 4], lhsT=m_win[:, t, :],
                                     rhs=wt4[:, j, :D + 4],
                                     start=(t == 0), stop=(t == T - 1))
            z = wp.tile([128, 4], F32, tag="z")
            nc.vector.tensor_scalar_add(z[:], ps_u[:, D:D + 4], 1e-30)
            rz = wp.tile([128, 4], F32, tag="rz")
            nc.vector.reciprocal(rz[:], z[:])
            aggt = wp.tile([128, D], F16, tag="aggt")
            nc.scalar.copy(out=aggt[:], in_=ps_u[:, :D])
            nc.vector.tensor_tensor(
                out=agg_sb[:, w, :].rearrange("p (h d) -> p h d", d=HD),
                in0=aggt[:].rearrange("p (h d) -> p h d", d=HD),
                in1=rz[:].to_broadcast([128, H, HD]), op=ALU.mult)

          # -- phase 4 (deferred): output projection + LN2 + relu + residual --
          # Runs after all windows so the scalar engine switches act-table
          # sets only twice per iteration (exp set <-> sqrt set).
          if "noB" in skips:
              return
          for w in range(nwin):
            aT = wp.tile([128, 2, 128], F16, tag="aT")
            for kh in range(2):
                pt = pp.tile([128, 128], F16, tag="psA")
                nc.tensor.transpose(out=pt[:],
                                    in_=agg_sb[:, w, kh * 128:(kh + 1) * 128],
                                    identity=ident[:])
                nc.scalar.copy(out=aT[:, kh, :], in_=pt[:])
            ps_o = pp.tile([128, D], F32, tag="psA")
            for kh in range(2):
                nc.tensor.matmul(ps_o[:], lhsT=aT[:, kh, :], rhs=wo_sb[:, kh, :],
                                 start=(kh == 0), stop=(kh == 1))
            o = wp.tile([128, D], F32, tag="o")
            if "bo" in bvec:
                nc.vector.tensor_add(o[:], ps_o[:], bvec["bo"][:])
            else:
                nc.scalar.copy(out=o[:], in_=ps_o[:])
            mean2 = ln_mean(o[:], "ln2")
            oc = wp.tile([128, D], F32, tag="oc")
            nc.vector.scalar_tensor_tensor(
                out=oc[:], in0=mean2[:, :1].to_broadcast([128, D]),
                scalar=-1.0 / D, op0=ALU.mult, in1=o[:], op1=ALU.add)
            rstd2 = ln_rstd(oc[:], "ln2")
            on = wp.tile([128, D], F32, tag="on")
            nc.vector.tensor_scalar_mul(on[:], oc[:], rstd2[:, :1])
            if "g2" in bvec:
                nc.vector.tensor_mul(on[:], on[:], bvec["g2"][:])
            if "b2" in bvec:
                nc.vector.tensor_add(on[:], on[:], bvec["b2"][:])
            r = wp.tile([128, D], F32, tag="r")
            nc.scalar.activation(out=r[:], in_=on[:], func=AF.Relu)
            yf = wp.tile([128, D], F32, tag="yf")
            nc.vector.tensor_add(yf[:], r[:], x_sb[:, w, :])
            nc.sync.dma_start(out=y_ap[w * 128:(w + 1) * 128, :], in_=yf[:])

        phase1()
        nc.gpsimd.collective_compute(
            "AllGather", ALU.bypass,
            replica_groups=[list(range(NCORE))],
            ins=[kv_local.ap().opt()], outs=[kv_shared.ap().opt()],
        )
        if bench:
            with tc.For_i(0, niter_reg, 1):
                if "phase1" not in skips:
                    phase1()
                if "coll" in skips:
                    nc.gpsimd.collective_compute(
                        "AllGather", ALU.bypass,
                        replica_groups=[list(range(NCORE))],
                        ins=[kv_local.ap().opt()], outs=[kv_shared.ap().opt()],
                    )
                if "phase34" not in skips:
                    phase34()
        else:
            phase34()

    nc.compile()
    return nc


_CACHE = {}


def kernel(x, edge_index, gamma1, beta1, gamma2, beta2,
           Wq, bq, Wk, bk, Wv, bv, Wo, bo):
    x = np.asarray(x, dtype=np.float32)
    edge_index = np.asarray(edge_index)
    n_nodes = x.shape[0]
    per_core, T, nwin, npad, npc = prep_inputs(x, edge_index, n_nodes)

    g1 = np.asarray(gamma1, np.float32)
    b1 = np.asarray(beta1, np.float32)
    wq_p = (g1[:, None] * np.asarray(Wq, np.float32)).astype(np.float16)
    wk_p = (g1[:, None] * np.asarray(Wk, np.float32)).astype(np.float16)
    wv_p = (g1[:, None] * np.asarray(Wv, np.float32)).astype(np.float16)
    wo_p = np.asarray(Wo, np.float32).astype(np.float16)
    bq_p = b1 @ np.asarray(Wq, np.float32) + np.asarray(bq, np.float32)
    bk_p = b1 @ np.asarray(Wk, np.float32) + np.asarray(bk, np.float32)
    bv_p = b1 @ np.asarray(Wv, np.float32) + np.asarray(bv, np.float32)
    bo_ = np.asarray(bo, np.float32)
    g2 = np.asarray(gamma2, np.float32)
    b2 = np.asarray(beta2, np.float32)
    vecs = np.stack([bq_p, bk_p, bv_p, bo_, g2, b2, np.zeros_like(g2),
                     np.zeros_like(g2)]).astype(np.float32)
    flags = dict(
        skip_bq=not bq_p.any(), skip_bk=not bk_p.any(), skip_bv=not bv_p.any(),
        skip_bo=not bo_.any(), skip_g2=bool((g2 == 1).all()),
        skip_b2=not b2.any(),
    )

    key = (T, nwin, npad, tuple(sorted(flags.items())))
    if key not in _CACHE:
        _CACHE[key] = build_program(T, nwin, npad, flags)
    nc = _CACHE[key]

    in_maps = []
    for c in range(NCORE):
        pc = per_core[c]
        in_maps.append(dict(
            x_pad=pc["x_pad"], wq=wq_p, wk=wk_p, wv=wv_p, wo=wo_p, vecs=vecs,
            kv_idx=pc["kv_idx"], dadj_col=pc["dadj_col"],
            dadj_row=pc["dadj_row"],
        ))
    res = run_bass_kernel_spmd(nc, in_maps, core_ids=list(range(NCORE)))
    out = np.concatenate([res.results[c]["y"][:npc] for c in range(NCORE)], axis=0)
    return out.astype(np.float32)


# revision 33
# speedup vs baseline: 1.5170x; 1.0201x over previous
# Graph-attention block (pre-LN, 4-head edge softmax, residual) on 8 Trainium2
# NeuronCores via Bass/Tile.
#
# Strategy (edge-cut partitioning per the sharding hint):
#   - Nodes are partitioned across the 8 cores by destination (1250 nodes/core,
#     padded to 1280 = 10 windows of 128).
#   - Each core computes LN1 + q/k/v projections for its own node slice; the
#     fp16 [k|v] rows are AllGathered so every core holds the full 10240x512
#     table, from which it bulk-gathers the source rows of its own edges
#     (descriptor-bound: ~7.6ns per gathered row).
#   - Edges are binned to the core owning their dst, sorted by dst, padded so
#     every (core, window) has the same tile count T. Per window both one-hot
#     orientations ([edge,dst] for the segment-sum matmul and [dst,edge] for
#     the q-expansion matmul) are built in single batched vector compares
#     against iota patterns - no per-tile transposes.
#   - Loop A (scores): per 4-tile supertile, 4 q-expansion matmuls into PSUM
#     quarters, one bulk scalar PSUM->SBUF drain, one batched q*k product and
#     a pairwise-add fold tree ending in a f32 reduce.
#   - Loop B (aggregation): batched exp-weighted v rows (+ the exp weights as
#     4 extra columns giving the softmax normalizer z) accumulated over the
#     window by the tensor engine; epilogue divides by z; output projection +
#     LN2 + ReLU + residual run inline per window.
import math
from contextlib import ExitStack

import numpy as np

import concourse.bass as bass
import concourse.tile as tile
from concourse import bacc, mybir
from concourse.bass_utils import run_bass_kernel_spmd
from concourse.masks import make_identity

F32 = mybir.dt.float32
F16 = mybir.dt.float16
I16 = mybir.dt.int16
I32 = mybir.dt.int32
AF = mybir.ActivationFunctionType
ALU = mybir.AluOpType
AX = mybir.AxisListType

EPS = 1e-5
D = 256
H = 4
HD = 64
NCORE = 8
REDUCE_MODE = "tree"   # "pool" | "tree"


def _cdiv(a, b):
    return (a + b - 1) // b


def prep_inputs(x, edge_index, n_nodes):
    """Host-side edge binning/sorting/padding. Returns per-core arrays + T."""
    npc = n_nodes // NCORE            # real nodes per core
    nwin = _cdiv(npc, 128)            # 128-node windows per core
    npad = nwin * 128                 # padded nodes per core
    src = np.asarray(edge_index[0], dtype=np.int64)
    dst = np.asarray(edge_index[1], dtype=np.int64)

    per_core = []
    tiles = np.zeros((NCORE, nwin), dtype=np.int64)
    for c in range(NCORE):
        m = (dst // npc) == c
        s = src[m]
        dl = dst[m] - c * npc
        order = np.argsort(dl, kind="stable")
        s, dl = s[order], dl[order]
        w = dl // 128
        cnt = np.bincount(w, minlength=nwin)
        tiles[c] = np.maximum(_cdiv(cnt, 128), 1)
        per_core.append((s, dl, cnt))
    T = int(tiles.max())

    out = []
    for c in range(NCORE):
        s, dl, cnt = per_core[c]
        ne = nwin * T * 128
        src_pad = np.zeros(ne, dtype=np.int64)
        dadj_pad = np.full(ne, -1.0, dtype=np.float16)
        base = np.concatenate([[0], np.cumsum(cnt)])
        for w in range(nwin):
            seg = slice(base[w], base[w + 1])
            k = cnt[w]
            o = w * T * 128
            src_pad[o:o + k] = s[seg]
            dadj_pad[o:o + k] = (dl[seg] - 128 * w).astype(np.float16)
        # global row index in the padded AllGather table
        gidx = ((src_pad // npc) * npad + src_pad % npc).astype(np.int16)
        # dma_gather idx layout: per window block, idx j -> [j%16, j//16], x8 replicated
        blocks = []
        for w in range(nwin):
            b = gidx[w * T * 128:(w + 1) * T * 128].reshape(T * 8, 16).T
            blocks.append(np.tile(b, (8, 1)))
        kv_idx = np.ascontiguousarray(np.concatenate(blocks, axis=1))
        # dadj per edge, edge-on-partition layout: [e%128, w*T + t]
        dadj_col = np.ascontiguousarray(
            dadj_pad.reshape(nwin * T, 128).T).astype(np.float16)
        # dadj per edge, row layout for partition_broadcast: [1, w*T*128 + e]
        dadj_row = dadj_pad.reshape(1, ne).astype(np.float16)
        xs = np.zeros((npad, D), dtype=np.float16)
        xs[:npc] = x[c * npc:(c + 1) * npc]
        out.append(dict(kv_idx=kv_idx, dadj_col=dadj_col, dadj_row=dadj_row,
                        x_pad=xs))
    return out, T, nwin, npad, npc


def build_program(T, nwin, npad, flags, bench=False, skips=()):
    """Build the SPMD Bass program. flags: dict of skip_* bools."""
    V = NCORE * npad
    nc = bacc.Bacc("TRN2", target_bir_lowering=False, debug=False,
                   num_devices=NCORE)

    # window split into two gather halves; supertile groups of up to 4 tiles
    # within each half (half boundary chosen 4-aligned so group count stays 9)
    H0 = (T // 2) // 4 * 4
    HALVES = [(0, H0), (H0, T - H0)]
    def _groups(n):
        return [(t0, min(4, n - t0)) for t0 in range(0, n, 4)]

    # ---- I/O ----
    x_ap = nc.dram_tensor("x_pad", [npad, D], F16, kind="ExternalInput").ap()
    wq_ap = nc.dram_tensor("wq", [D, D], F16, kind="ExternalInput").ap()
    wk_ap = nc.dram_tensor("wk", [D, D], F16, kind="ExternalInput").ap()
    wv_ap = nc.dram_tensor("wv", [D, D], F16, kind="ExternalInput").ap()
    wo_ap = nc.dram_tensor("wo", [D, D], F16, kind="ExternalInput").ap()
    vec_ap = nc.dram_tensor("vecs", [8, D], F32, kind="ExternalInput").ap()
    # vecs rows: 0:bq', 1:bk', 2:bv', 3:bo, 4:gamma2, 5:beta2 (fp32)
    kvidx_ap = nc.dram_tensor("kv_idx", [128, nwin * T * 8], I16,
                              kind="ExternalInput").ap()
    dadjc_ap = nc.dram_tensor("dadj_col", [128, nwin * T], F16,
                              kind="ExternalInput").ap()
    dadjr_ap = nc.dram_tensor("dadj_row", [1, nwin * T * 128], F16,
                              kind="ExternalInput").ap()
    y_ap = nc.dram_tensor("y", [npad, D], F32, kind="ExternalOutput").ap()
    n_ap = (nc.dram_tensor("niter", [1, 1], I32, kind="ExternalInput").ap()
            if bench else None)

    kv_local = nc.dram_tensor("kv_local", [npad, 2 * D], F16)
    kv_shared = nc.dram_tensor("kv_shared", [V, 2 * D], F16, addr_space="Shared")

    with tile.TileContext(nc) as tc, ExitStack() as ctx:
        cp = ctx.enter_context(tc.tile_pool(name="const", bufs=1))
        wp = ctx.enter_context(tc.tile_pool(name="work", bufs=2))
        mp = ctx.enter_context(tc.tile_pool(name="mask", bufs=2))
        bp = ctx.enter_context(tc.tile_pool(name="bcast", bufs=1))
        gp = ctx.enter_context(tc.tile_pool(name="gath", bufs=2))
        pp = ctx.enter_context(tc.tile_pool(name="ps", bufs=2, space="PSUM"))
        up = ctx.enter_context(tc.tile_pool(name="psu", bufs=2, space="PSUM"))

        # ---- constants ----
        ident = cp.tile([128, 128], F16)
        make_identity(nc, ident[:])
        # stage the int iota in a gather-pool ring buffer (reused later)
        ii = gp.tile([128, T * 128], I16, tag="kvg")
        nc.gpsimd.iota(ii[:], pattern=[[0, T], [1, 128]], channel_multiplier=0)
        iota_col = cp.tile([128, T * 128], F16)
        nc.vector.tensor_copy(iota_col[:], ii[:])
        ip = cp.tile([128, 1], I16)
        nc.gpsimd.iota(ip[:], pattern=[[0, 1]], channel_multiplier=1)
        iota_part = cp.tile([128, 1], F16)
        nc.vector.tensor_copy(iota_part[:], ip[:])
        eps_sb = cp.tile([128, 1], F32)
        nc.gpsimd.memset(eps_sb[:], EPS)

        wq_sb = cp.tile([128, 2, D], F16)
        wk_sb = cp.tile([128, 2, D], F16)
        wv_sb = cp.tile([128, 2, D], F16)
        wo_sb = cp.tile([128, 2, D], F16)
        for w_ap, w_sb in ((wq_ap, wq_sb), (wk_ap, wk_sb), (wv_ap, wv_sb),
                           (wo_ap, wo_sb)):
            nc.sync.dma_start(out=w_sb[:],
                              in_=w_ap.rearrange("(b k) n -> k b n", k=128))
        vec_sb = cp.tile([8, D], F32)
        nc.sync.dma_start(out=vec_sb[:], in_=vec_ap[:, :])
        bvec = {}
        for name, row in (("bq", 0), ("bk", 1), ("bv", 2), ("bo", 3),
                          ("g2", 4), ("b2", 5)):
            if not flags.get("skip_" + name, False):
                t = cp.tile([128, D], F32, tag="bc_" + name)
                nc.gpsimd.partition_broadcast(t[:], vec_sb[row:row + 1, :])
                bvec[name] = t

        kvidx_sb = cp.tile([128, nwin * T * 8], I16)
        nc.sync.dma_start(out=kvidx_sb[:], in_=kvidx_ap[:, :])
        dadjc_sb = cp.tile([128, nwin * T], F16)
        nc.sync.dma_start(out=dadjc_sb[:], in_=dadjc_ap[:, :])

        if bench:
            nn_t = cp.tile([1, 1], I32)
            nc.sync.dma_start(out=nn_t[:], in_=n_ap[:, :])
        x_sb = cp.tile([128, nwin, D], F16)
        q_sb = cp.tile([128, nwin, D], F16)
        agg_sb = cp.tile([128, nwin, D], F16)

        niter_reg = (nc.values_load(nn_t[:1, :1], min_val=0, max_val=1000000,
                                    skip_runtime_bounds_check=True)
                     if bench else None)

        def ln_mean(x_ap, tag):
            """Row-sum via scalar Copy+accum (no act-table switch)."""
            ms = wp.tile([128, D], F16, tag=tag + "_ms")
            mean = wp.tile([128, 1], F32, tag=tag + "_m")
            nc.scalar.activation(out=ms[:], in_=x_ap, func=AF.Copy,
                                 accum_out=mean[:])
            return mean

        def ln_rstd(xc_ap, tag):
            """rstd of centered rows [128, D]; scalar Square+Sqrt (both live
            in the sqrt act-table set together with Copy/Relu)."""
            sq = wp.tile([128, D], F16, tag=tag + "_sq")
            var = wp.tile([128, 1], F32, tag=tag + "_v")
            nc.scalar.activation(out=sq[:], in_=xc_ap, func=AF.Square,
                                 accum_out=var[:])
            s = wp.tile([128, 1], F32, tag=tag + "_s")
            nc.scalar.activation(out=s[:], in_=var[:], func=AF.Sqrt,
                                 scale=1.0 / D, bias=eps_sb[:, :1])
            rstd = wp.tile([128, 1], F32, tag=tag + "_r")
            nc.vector.reciprocal(rstd[:], s[:])
            return rstd

        # ---- phase 1: LN1 + projections on own slice ----
        def phase1():
          for w in range(nwin):
            xw = x_sb[:, w, :]
            nc.sync.dma_start(out=xw, in_=x_ap[w * 128:(w + 1) * 128, :])
            mean = ln_mean(xw, "ln1")
            xc = wp.tile([128, D], F32, tag="xc")
            nc.vector.scalar_tensor_tensor(
                out=xc[:], in0=mean[:, :1].to_broadcast([128, D]),
                scalar=-1.0 / D, op0=ALU.mult, in1=xw, op1=ALU.add)
            rstd = ln_rstd(xc[:], "ln1")
            xn = wp.tile([128, D], F16, tag="xn")
            nc.vector.tensor_scalar_mul(xn[:], xc[:], rstd[:, :1])

            xnT = wp.tile([128, 2, 128], F16, tag="xnT")
            for kh in range(2):
                pt = pp.tile([128, 128], F16, tag="psA")
                nc.tensor.transpose(out=pt[:], in_=xn[:, kh * 128:(kh + 1) * 128],
                                    identity=ident[:])
                nc.scalar.copy(out=xnT[:, kh, :], in_=pt[:])

            kv16 = wp.tile([128, 2 * D], F16, tag="kv16")
            for name, w_sb_, dst in (("bq", wq_sb, None), ("bk", wk_sb, kv16[:, :D]),
                                     ("bv", wv_sb, kv16[:, D:])):
                ps = pp.tile([128, D], F32, tag="psA")
                for kh in range(2):
                    nc.tensor.matmul(ps[:], lhsT=xnT[:, kh, :],
                                     rhs=w_sb_[:, kh, :],
                                     start=(kh == 0), stop=(kh == 1))
                tgt = q_sb[:, w, :] if dst is None else dst
                if name in bvec:
                    tf = wp.tile([128, D], F32, tag="pbias")
                    nc.vector.tensor_add(tf[:], ps[:], bvec[name][:])
                    nc.scalar.copy(out=tgt, in_=tf[:])
                else:
                    nc.scalar.copy(out=tgt, in_=ps[:])
            nc.sync.dma_start(out=kv_local[w * 128:(w + 1) * 128, :], in_=kv16[:])

        def phase34():
          for w in range(nwin):
            # -- per-window inputs (DMA / pool engine) --
            # gathers FIRST on the in-order Pool queue so they are never
            # stalled behind a broadcast that waits on DVE mask builds
            kv_g = gp.tile([128, T, 2 * D], F16, tag="kvg")
            n_idx = 128 if "gather" in skips else T * 128
            nc.gpsimd.dma_gather(
                out_ap=kv_g[:, :_cdiv(n_idx, 128), :],
                in_ap=kv_shared.ap()[:, :],
                idxs_ap=kvidx_sb[:, w * T * 8:w * T * 8 + n_idx // 16],
                num_idxs=n_idx, num_idxs_reg=n_idx, elem_size=2 * D,
                single_packet=False)
            kv_h = [kv_g[:, o:o + n, :] for (o, n) in HALVES]
            dr = bp.tile([1, T * 128], F16, tag="dr")
            nc.sync.dma_start(
                out=dr[:], in_=dadjr_ap[:1, w * T * 128:(w + 1) * T * 128])
            dadj_bc = bp.tile([128, T * 128], F16, tag="bc")
            nc.gpsimd.partition_broadcast(dadj_bc[:], dr[:1, :])
            # -- one-hot masks, both orientations, batched --
            m_win = mp.tile([128, T, 128], F16, tag="mw")
            nc.vector.tensor_tensor(
                out=m_win[:],
                in0=dadjc_sb[:, w * T:(w + 1) * T].to_broadcast([128, T, 128]),
                in1=iota_col[:].rearrange("p (t j) -> p t j", j=128),
                op=ALU.is_equal)
            mt = mp.tile([128, T * 128], F16, tag="mt")
            nc.vector.tensor_tensor(
                out=mt[:], in0=dadj_bc[:],
                in1=iota_part[:].to_broadcast([128, T * 128]), op=ALU.is_equal)

            scores = wp.tile([128, T * 4], F32, tag="sc")
            if "noA" in skips:
                nc.vector.memset(scores[:], 0.0)
            else:
              for hi, (off, nt_h) in enumerate(HALVES):
               for (tl, nt) in _groups(nt_h):
                t0 = off + tl
                ps_qe = pp.tile([128, 4, D], F32, tag="psQ")
                for j in range(nt):
                    nc.tensor.matmul(ps_qe[:, j, :],
                                     lhsT=mt[:, (t0 + j) * 128:(t0 + j + 1) * 128],
                                     rhs=q_sb[:, w, :], start=True, stop=True)
                qe16 = wp.tile([128, 4, D], F16, tag="qe16")
                nc.scalar.copy(out=qe16[:, :nt, :], in_=ps_qe[:, :nt, :])
                prod = wp.tile([128, 4, D], F16, tag="prod")
                nc.vector.tensor_tensor(
                    out=prod[:, :nt, :], in0=qe16[:, :nt, :],
                    in1=kv_h[hi][:, tl:tl + nt, :D], op=ALU.mult)
                pv = prod[:].rearrange("p t (h d) -> p (t h) d", d=HD)
                nh = nt * 4
                if REDUCE_MODE == "pool":
                    nc.vector.pool(out=scores[:, t0 * 4:t0 * 4 + nh],
                                   in_=pv[:, :nh, :],
                                   func=mybir.PoolFunctionType.avg)
                elif REDUCE_MODE == "gp":
                    nc.gpsimd.reduce_sum(
                        out=scores[:, t0 * 4:t0 * 4 + nh].rearrange(
                            "p (th one) -> p th one", one=1),
                        in_=pv[:, :nh, :], axis=AX.X)
                else:
                    f1 = wp.tile([128, 16, 32], F16, tag="f1")
                    nc.vector.tensor_tensor(out=f1[:, :nh, :],
                                            in0=pv[:, :nh, 0:32],
                                            in1=pv[:, :nh, 32:64], op=ALU.add)
                    f2 = wp.tile([128, 16, 16], F16, tag="f2")
                    nc.vector.tensor_tensor(out=f2[:, :nh, :],
                                            in0=f1[:, :nh, 0:16],
                                            in1=f1[:, :nh, 16:32], op=ALU.add)
                    f3 = wp.tile([128, 16, 8], F16, tag="f3")
                    nc.vector.tensor_tensor(out=f3[:, :nh, :],
                                            in0=f2[:, :nh, 0:8],
                                            in1=f2[:, :nh, 8:16], op=ALU.add)
                    nc.vector.reduce_sum(
                        out=scores[:, t0 * 4:t0 * 4 + nh].rearrange(
                            "p (th one) -> p th one", one=1),
                        in_=f3[:, :nh, :], axis=AX.X)
            e_s = wp.tile([128, T * 4], F16, tag="es")
            # pool averages over HD; fold the *HD back into the exp scale
            es_scale = (float(HD) if REDUCE_MODE == "pool" else 1.0) / math.sqrt(HD)
            nc.scalar.activation(out=e_s[:], in_=scores[:], func=AF.Exp,
                                 scale=es_scale)

            ps_u = up.tile([128, D + 8], F32, tag="u")
            if "noB" in skips:
                continue
            for hi, (off, nt_h) in enumerate(HALVES):
              for (tl, nt) in _groups(nt_h):
                t0 = off + tl
                wt4 = wp.tile([128, 4, D + 8], F16, tag="wt")
                nc.vector.tensor_tensor(
                    out=wt4[:, :nt, :D].rearrange("p t (h d) -> p t h d", d=HD),
                    in0=kv_h[hi][:, tl:tl + nt, D:].rearrange(
                        "p t (h d) -> p t h d", d=HD),
                    in1=e_s[:, t0 * 4:(t0 + nt) * 4].rearrange(
                        "p (t h) -> p t h", h=4).to_broadcast([128, nt, 4, HD]),
                    op=ALU.mult)
                nc.vector.tensor_copy(
                    wt4[:, :nt, D:D + 4],
                    e_s[:, t0 * 4:(t0 + nt) * 4].rearrange("p (t h) -> p t h", h=4))
                for j in range(nt):
                    t = t0 + j
                    nc.tensor.matmul(ps_u[:, :D + 4], lhsT=m_win[:, t, :],
                                     rhs=wt4[:, j, :D + 4],
                                     start=(t == 0), stop=(t == T - 1))
            z = wp.tile([128, 4], F32, tag="z")
            nc.vector.tensor_scalar_add(z[:], ps_u[:, D:D + 4], 1e-30)
            rz = wp.tile([128, 4], F32, tag="rz")
            nc.vector.reciprocal(rz[:], z[:])
            aggt = wp.tile([128, D], F16, tag="aggt")
            nc.scalar.copy(out=aggt[:], in_=ps_u[:, :D])
            nc.vector.tensor_tensor(
                out=agg_sb[:, w, :].rearrange("p (h d) -> p h d", d=HD),
                in0=aggt[:].rearrange("p (h d) -> p h d", d=HD),
                in1=rz[:].to_broadcast([128, H, HD]), op=ALU.mult)

          # -- phase 4 (deferred): output projection + LN2 + relu + residual --
          # Runs after all windows so the scalar engine switches act-table
          # sets only twice per iteration (exp set <-> sqrt set).
          if "noB" in skips:
              return
          for w in range(nwin):
            aT = wp.tile([128, 2, 128], F16, tag="aT")
            for kh in range(2):
                pt = pp.tile([128, 128], F16, tag="psA")
                nc.tensor.transpose(out=pt[:],
                                    in_=agg_sb[:, w, kh * 128:(kh + 1) * 128],
                                    identity=ident[:])
                nc.scalar.copy(out=aT[:, kh, :], in_=pt[:])
            ps_o = pp.tile([128, D], F32, tag="psA")
            for kh in range(2):
                nc.tensor.matmul(ps_o[:], lhsT=aT[:, kh, :], rhs=wo_sb[:, kh, :],
                                 start=(kh == 0), stop=(kh == 1))
            o = wp.tile([128, D], F32, tag="o")
            if "bo" in bvec:
                nc.vector.tensor_add(o[:], ps_o[:], bvec["bo"][:])
            else:
                nc.scalar.copy(out=o[:], in_=ps_o[:])
            mean2 = ln_mean(o[:], "ln2")
            oc = wp.tile([128, D], F32, tag="oc")
            nc.vector.scalar_tensor_tensor(
                out=oc[:], in0=mean2[:, :1].to_broadcast([128, D]),
                scalar=-1.0 / D, op0=ALU.mult, in1=o[:], op1=ALU.add)
            rstd2 = ln_rstd(oc[:], "ln2")
            on = wp.tile([128, D], F32, tag="on")
            nc.vector.tensor_scalar_mul(on[:], oc[:], rstd2[:, :1])
            if "g2" in bvec:
                nc.vector.tensor_mul(on[:], on[:], bvec["g2"][:])
            if "b2" in bvec:
                nc.vector.tensor_add(on[:], on[:], bvec["b2"][:])
            r = wp.tile([128, D], F32, tag="r")
            nc.scalar.activation(out=r[:], in_=on[:], func=AF.Relu)
            yf = wp.tile([128, D], F32, tag="yf")
            nc.vector.tensor_add(yf[:], r[:], x_sb[:, w, :])
            nc.sync.dma_start(out=y_ap[w * 128:(w + 1) * 128, :], in_=yf[:])

        phase1()
        nc.gpsimd.collective_compute(
            "AllGather", ALU.bypass,
            replica_groups=[list(range(NCORE))],
            ins=[kv_local.ap().opt()], outs=[kv_shared.ap().opt()],
        )
        if bench:
            with tc.For_i(0, niter_reg, 1):
                if "phase1" not in skips:
                    phase1()
                if "coll" in skips:
                    nc.gpsimd.collective_compute(
                        "AllGather", ALU.bypass,
                        replica_groups=[list(range(NCORE))],
                        ins=[kv_local.ap().opt()], outs=[kv_shared.ap().opt()],
                    )
                if "phase34" not in skips:
                    phase34()
        else:
            phase34()

    nc.compile()
    return nc


_CACHE = {}


def kernel(x, edge_index, gamma1, beta1, gamma2, beta2,
           Wq, bq, Wk, bk, Wv, bv, Wo, bo):
    x = np.asarray(x, dtype=np.float32)
    edge_index = np.asarray(edge_index)
    n_nodes = x.shape[0]
    per_core, T, nwin, npad, npc = prep_inputs(x, edge_index, n_nodes)

    g1 = np.asarray(gamma1, np.float32)
    b1 = np.asarray(beta1, np.float32)
    wq_p = (g1[:, None] * np.asarray(Wq, np.float32)).astype(np.float16)
    wk_p = (g1[:, None] * np.asarray(Wk, np.float32)).astype(np.float16)
    wv_p = (g1[:, None] * np.asarray(Wv, np.float32)).astype(np.float16)
    wo_p = np.asarray(Wo, np.float32).astype(np.float16)
    bq_p = b1 @ np.asarray(Wq, np.float32) + np.asarray(bq, np.float32)
    bk_p = b1 @ np.asarray(Wk, np.float32) + np.asarray(bk, np.float32)
    bv_p = b1 @ np.asarray(Wv, np.float32) + np.asarray(bv, np.float32)
    bo_ = np.asarray(bo, np.float32)
    g2 = np.asarray(gamma2, np.float32)
    b2 = np.asarray(beta2, np.float32)
    vecs = np.stack([bq_p, bk_p, bv_p, bo_, g2, b2, np.zeros_like(g2),
                     np.zeros_like(g2)]).astype(np.float32)
    flags = dict(
        skip_bq=not bq_p.any(), skip_bk=not bk_p.any(), skip_bv=not bv_p.any(),
        skip_bo=not bo_.any(), skip_g2=bool((g2 == 1).all()),
        skip_b2=not b2.any(),
    )

    key = (T, nwin, npad, tuple(sorted(flags.items())))
    if key not in _CACHE:
        _CACHE[key] = build_program(T, nwin, npad, flags)
    nc = _CACHE[key]

    in_maps = []
    for c in range(NCORE):
        pc = per_core[c]
        in_maps.append(dict(
            x_pad=pc["x_pad"], wq=wq_p, wk=wk_p, wv=wv_p, wo=wo_p, vecs=vecs,
            kv_idx=pc["kv_idx"], dadj_col=pc["dadj_col"],
            dadj_row=pc["dadj_row"],
        ))
    res = run_bass_kernel_spmd(nc, in_maps, core_ids=list(range(NCORE)))
    out = np.concatenate([res.results[c]["y"][:npc] for c in range(NCORE)], axis=0)
    return out.astype(np.float32)
